# revision 23
# baseline (speedup 1.0000x reference)
"""Trainium2 Bass kernel for the sparse-attention nn module (nn_BDH_48421461295735).

Strategy: 8 NeuronCores = 8 (batch, head) pairs (B=2 x NH=4).  Each core runs
all 4 layers for its head; the only cross-core traffic is a 4-core AllReduce
per layer of the per-head decoder partial p = (x_sparse*y_sparse) @ dec_h,
done within each batch's group of 4 cores.

Layouts (per core):
  x        [T, D]   f32 master (+ bf16 copy and bf16 transposed copy xT [D, T])
  qrT      [N, T]   bf16 resident: rope(relu(enc^T x^T)); neuron axis is
                    host-permuted (evens then odds) so the rope pair partner
                    of partition-tile j is partition-tile j+nJ (no shuffles).
  scores   computed tile-wise [128 s, 512 t] = qrT^T qrT, strictly-causal
           masked, only the lower triangle of (s,t) blocks is computed.
  ykv      [128 t, D] psum accum -> LN -> transposed into ykvT [D, T] bf16.
  y_sparseT/x_sparseT recomputed tile-wise, xy = prod, p accum [128 t, D].

Wait-slot discipline: trn2 TPB instructions hold at most 2 sem waits.  Within
a phase every instruction's deps span <=2 procs by construction (relu/evac on
DVE so slot-WAR merges with data deps).  At phase boundaries, per-engine
Drain instructions (which may carry many waits) fan in the previous phase's
cross-engine tails, advancing each engine's observed vector clock so
steady-state instructions need no cross-phase waits.  no_sync_barrier pins
the schedule order around the drains.
"""

import math
import sys

import numpy as np

for _p in ("/opt/trn_rl_repo",):
    if _p not in sys.path:
        sys.path.insert(0, _p)

import concourse.bass as bass
import concourse.bacc as bacc
import concourse.mybir as mybir
import concourse.tile as tile
from concourse.bass_utils import run_bass_kernel_spmd

F32 = mybir.dt.float32
BF16 = mybir.dt.bfloat16
AF = mybir.ActivationFunctionType
ALU = mybir.AluOpType

FULL_CFG = dict(T=2048, D=256, N=2048, NL=4, V=256, NH=4, B=2)
P = 128
SUP = 512
EPS = 1e-5


def build_nc(cfg, mm_dt=BF16, n_cores=8):
    T, D, N, NL, V = cfg["T"], cfg["D"], cfg["N"], cfg["NL"], cfg["V"]
    NH = cfg["NH"]
    assert T % SUP == 0 and D % P == 0 and N % 256 == 0 and V == D
    nTB, nTS, nD, nK = T // P, T // SUP, D // P, N // P
    nJ = nK // 2
    nQ = SUP // P  # 4

    nc = bacc.Bacc("TRN2", target_bir_lowering=False, debug=False,
                   num_devices=n_cores)

    x0_d = nc.dram_tensor("x0", [T, D], F32, kind="ExternalInput")
    wenc_d = nc.dram_tensor("wenc", [D, N], mm_dt, kind="ExternalInput")
    wencv_d = nc.dram_tensor("wencv", [D, N], mm_dt, kind="ExternalInput")
    wdec_d = nc.dram_tensor("wdec", [N, D], mm_dt, kind="ExternalInput")
    wlm_d = nc.dram_tensor("wlm", [D, V], mm_dt, kind="ExternalInput")
    ctab_d = nc.dram_tensor("ctab", [N // 2, T], mm_dt, kind="ExternalInput")
    stab_d = nc.dram_tensor("stab", [N // 2, T], mm_dt, kind="ExternalInput")
    ident_d = nc.dram_tensor("ident", [P, P], mm_dt, kind="ExternalInput")
    maskt_d = nc.dram_tensor("maskt", [P, P], mm_dt, kind="ExternalInput")
    out_d = nc.dram_tensor("out", [T, V], F32, kind="ExternalOutput")

    # AllReduce groups: one group of NH cores per batch.
    RG = [list(range(g * NH, (g + 1) * NH)) for g in range(n_cores // NH)]

    ET = mybir.EngineType
    COMPUTE_ENGINES = (ET.PE, ET.DVE, ET.Activation, ET.SP, ET.Pool)

    with tile.TileContext(nc) as tc:
        _keep = []  # keep tc.tile free-closures alive (GC would release pools)

        def ptile(shape, dtype, name, **kw):
            t, free = tc.tile(shape, dtype, name=name, **kw)
            _keep.append(free)
            return t

        # ---- persistent SBUF tensors ----
        wenc_sb = [ptile([P, N], mm_dt, name=f"wenc{d}") for d in range(nD)]
        wencv_sb = [ptile([P, N], mm_dt, name=f"wencv{d}") for d in range(nD)]
        wdec_sb = [ptile([P, D], mm_dt, name=f"wdec{k}") for k in range(nK)]
        wlm_sb = [ptile([P, V], mm_dt, name=f"wlm{d}") for d in range(nD)]
        ident_sb = ptile([P, P], mm_dt, name="ident")
        maskt_sb = ptile([P, P], mm_dt, name="maskt")
        x_f32 = [ptile([P, D], F32, name=f"xf{t}") for t in range(nTB)]
        x_bf = [ptile([P, D], mm_dt, name=f"xb{t}") for t in range(nTB)]
        xT_bf = [ptile([P, T], mm_dt, name=f"xT{d}") for d in range(nD)]
        qrT = [ptile([P, T], mm_dt, name=f"qrT{k}") for k in range(nK)]
        ykvT = [ptile([P, T], mm_dt, name=f"ykvT{d}") for d in range(nD)]
        p_stage = ptile([P, nTB * D], F32, name="pstage")
        p_in = ptile([P, nTB * D], F32, name="pin_stage")
        eps_sb = ptile([P, 1], F32, name="epsb")
        nc.vector.memset(eps_sb[:], EPS)

        # per-layer DRAM bounce buffers for the head-partial AllReduce
        p_loc = [ptile([T, D], F32, space="DRAM", name=f"ploc{l}")
                 for l in range(NL)]
        p_sum = [ptile([T, D], F32, space="DRAM", addr_space="Shared",
                       name=f"psum{l}") for l in range(NL)]

        # transient pools (keep the context managers alive until the end)
        _cms = [tc.tile_pool(name="sp2", bufs=2),
                tc.tile_pool(name="sp4", bufs=4),
                tc.tile_pool(name="spT", bufs=4),
                tc.tile_pool(name="ppbig", bufs=4, space="PSUM"),
                tc.tile_pool(name="ppsmall", bufs=4, space="PSUM")]
        sp2, sp4, spT, ppb, pps = [cm.__enter__() for cm in _cms]

        def layer_norm(src_ap, outs):
            """LN over free dim D of a [P, D] f32 AP; writes each AP in outs."""
            s1 = sp2.tile([P, 1], F32, tag="ln1", name="s1")
            nc.vector.reduce_sum(s1[:], src_ap, axis=mybir.AxisListType.X)
            nm = sp2.tile([P, 1], F32, tag="ln2", name="nm")
            nc.vector.tensor_scalar_mul(nm[:], s1[:], -1.0 / D)
            xc = sp2.tile([P, D], F32, tag="lnc", name="xc")
            nc.vector.tensor_scalar_add(xc[:], src_ap, nm[:])
            sq = sp2.tile([P, D], BF16, tag="lnsq", name="sq")
            ss = sp2.tile([P, 1], F32, tag="ln3", name="ss")
            nc.scalar.activation(sq[:], xc[:], AF.Square, accum_out=ss[:])
            sd = sp2.tile([P, 1], F32, tag="ln4", name="sd")
            nc.scalar.activation(sd[:], ss[:], AF.Sqrt, bias=eps_sb[:],
                                 scale=1.0 / D)
            rs = sp2.tile([P, 1], F32, tag="ln5", name="rs")
            nc.vector.reciprocal(rs[:], sd[:])
            for o in outs:
                nc.vector.tensor_scalar_mul(o, xc[:], rs[:])

        # ---- setup: load weights, x0; build x_bf and xT ----
        with nc.named_scope("setup"):
            for d in range(nD):
                nc.sync.dma_start(wenc_sb[d][:], wenc_d[d * P:(d + 1) * P, :])
                nc.sync.dma_start(wencv_sb[d][:], wencv_d[d * P:(d + 1) * P, :])
                nc.sync.dma_start(wlm_sb[d][:], wlm_d[d * P:(d + 1) * P, :])
            for k in range(nK):
                nc.sync.dma_start(wdec_sb[k][:], wdec_d[k * P:(k + 1) * P, :])
            nc.sync.dma_start(ident_sb[:], ident_d[:, :])
            nc.sync.dma_start(maskt_sb[:], maskt_d[:, :])
            for t in range(nTB):
                nc.sync.dma_start(x_f32[t][:], x0_d[t * P:(t + 1) * P, :])
                nc.vector.tensor_copy(x_bf[t][:], x_f32[t][:])
                for d in range(nD):
                    trp = pps.tile([P, P], mm_dt, tag="small", name="trp")
                    nc.tensor.transpose(trp[:], x_bf[t][:, d * P:(d + 1) * P],
                                        ident_sb[:])
                    nc.vector.tensor_copy(xT_bf[d][:, t * P:(t + 1) * P],
                                          trp[:])

        for l in range(NL):
            # ---- phase 1: qrT = rope(relu(enc^T x^T)) ----
            with nc.named_scope(f"l{l}_p1"):
                for j in range(nJ):
                    for ts in range(nTS):
                        c0, c1 = ts * SUP, (ts + 1) * SUP
                        ct = spT.tile([P, SUP], mm_dt, tag="ctc", name="ct")
                        st = spT.tile([P, SUP], mm_dt, tag="stc", name="st")
                        nc.sync.dma_start(ct[:], ctab_d[j * P:(j + 1) * P, c0:c1])
                        nc.sync.dma_start(st[:], stab_d[j * P:(j + 1) * P, c0:c1])
                        psA = ppb.tile([P, SUP], F32, tag="big", name="psA")
                        psB = ppb.tile([P, SUP], F32, tag="big", name="psB")
                        for d in range(nD):
                            nc.tensor.matmul(
                                psA[:], wenc_sb[d][:, j * P:(j + 1) * P],
                                xT_bf[d][:, c0:c1],
                                start=(d == 0), stop=(d == nD - 1))
                        for d in range(nD):
                            nc.tensor.matmul(
                                psB[:],
                                wenc_sb[d][:, (j + nJ) * P:(j + nJ + 1) * P],
                                xT_bf[d][:, c0:c1],
                                start=(d == 0), stop=(d == nD - 1))
                        xsA = sp4.tile([P, SUP], mm_dt, tag="xs", name="xsA")
                        xsB = sp4.tile([P, SUP], mm_dt, tag="xs", name="xsB")
                        nc.vector.tensor_relu(xsA[:], psA[:])
                        nc.vector.tensor_relu(xsB[:], psB[:])
                        t0 = sp4.tile([P, SUP], mm_dt, tag="rt", name="t0")
                        t1 = sp4.tile([P, SUP], mm_dt, tag="rt", name="t1")
                        nc.vector.tensor_tensor(t0[:], xsA[:], ct[:],
                                                ALU.mult)
                        nc.vector.tensor_tensor(t1[:], xsB[:], st[:],
                                                ALU.mult)
                        nc.vector.tensor_tensor(qrT[j][:, c0:c1], t0[:], t1[:],
                                                ALU.subtract)
                        t2 = sp4.tile([P, SUP], mm_dt, tag="rt", name="t2")
                        t3 = sp4.tile([P, SUP], mm_dt, tag="rt", name="t3")
                        nc.vector.tensor_tensor(t2[:], xsB[:], ct[:],
                                                ALU.mult)
                        nc.vector.tensor_tensor(t3[:], xsA[:], st[:],
                                                ALU.mult)
                        nc.vector.tensor_tensor(qrT[j + nJ][:, c0:c1], t2[:],
                                                t3[:], ALU.add)

            # ---- phase 2: scores -> ykv -> LN -> ykvT ----
            with nc.named_scope(f"l{l}_p2"):
                for ts in range(nTS):
                    c0 = ts * SUP
                    ykv_ps = [pps.tile([P, D], F32, tag="small", name=f"ykv{q}")
                              for q in range(nQ)]
                    for sb in range(nQ * ts + nQ):
                        r = sb - nQ * ts  # q index this s-block is diagonal with
                        q0 = max(0, r)
                        st_ps = ppb.tile([P, SUP], F32, tag="big", name="st_ps")
                        dst = st_ps[:, q0 * P:SUP]
                        for k in range(nK):
                            nc.tensor.matmul(
                                dst, qrT[k][:, sb * P:(sb + 1) * P],
                                qrT[k][:, c0 + q0 * P:c0 + SUP],
                                start=(k == 0), stop=(k == nK - 1))
                        st_sb = sp4.tile([P, SUP], mm_dt, tag="stsb",
                                         name="st_sb")
                        if r >= 0:
                            nc.vector.tensor_tensor(
                                st_sb[:, r * P:(r + 1) * P],
                                st_ps[:, r * P:(r + 1) * P], maskt_sb[:],
                                ALU.mult)
                            if r + 1 < nQ:
                                nc.vector.tensor_copy(
                                    st_sb[:, (r + 1) * P:SUP],
                                    st_ps[:, (r + 1) * P:SUP])
                        else:
                            nc.vector.tensor_copy(st_sb[:], st_ps[:])
                        for q in range(q0, nQ):
                            nc.tensor.matmul(
                                ykv_ps[q][:], st_sb[:, q * P:(q + 1) * P],
                                x_bf[sb][:],
                                start=(sb == 0), stop=(sb == nQ * ts + q))
                    for q in range(nQ):
                        tb = nQ * ts + q
                        ykv_n = sp2.tile([P, D], mm_dt, tag="ykvn",
                                         name="ykv_n")
                        layer_norm(ykv_ps[q][:], [ykv_n[:]])
                        for d in range(nD):
                            trp = pps.tile([P, P], mm_dt, tag="small",
                                           name="trp2")
                            nc.tensor.transpose(trp[:],
                                                ykv_n[:, d * P:(d + 1) * P],
                                                ident_sb[:])
                            nc.vector.tensor_copy(
                                ykvT[d][:, tb * P:(tb + 1) * P], trp[:])

            # ---- phase 3: y_sparse, x_sparse recompute, xy, p partial ----
            with nc.named_scope(f"l{l}_p3"):
                for ts in range(nTS):
                    c0, c1 = ts * SUP, (ts + 1) * SUP
                    p_ps = [pps.tile([P, D], F32, tag="small", name=f"pp{q}")
                            for q in range(nQ)]
                    for k in range(nK):
                        ys_ps = ppb.tile([P, SUP], F32, tag="big", name="ys_ps")
                        xs_ps = ppb.tile([P, SUP], F32, tag="big", name="xs_ps")
                        for d in range(nD):
                            nc.tensor.matmul(
                                ys_ps[:], wencv_sb[d][:, k * P:(k + 1) * P],
                                ykvT[d][:, c0:c1],
                                start=(d == 0), stop=(d == nD - 1))
                        for d in range(nD):
                            nc.tensor.matmul(
                                xs_ps[:], wenc_sb[d][:, k * P:(k + 1) * P],
                                xT_bf[d][:, c0:c1],
                                start=(d == 0), stop=(d == nD - 1))
                        ysr = sp4.tile([P, SUP], mm_dt, tag="ysx", name="ysr")
                        xsr = sp4.tile([P, SUP], mm_dt, tag="ysx", name="xsr")
                        nc.vector.tensor_relu(ysr[:], ys_ps[:])
                        nc.vector.tensor_relu(xsr[:], xs_ps[:])
                        xy = sp4.tile([P, SUP], mm_dt, tag="xy", name="xy")
                        nc.vector.tensor_tensor(xy[:], ysr[:], xsr[:],
                                                ALU.mult)
                        for q in range(nQ):
                            nc.tensor.matmul(
                                p_ps[q][:], xy[:, q * P:(q + 1) * P],
                                wdec_sb[k][:],
                                start=(k == 0), stop=(k == nK - 1))
                    for q in range(nQ):
                        tb = nQ * ts + q
                        nc.vector.tensor_copy(
                            p_stage[:, tb * D:(tb + 1) * D], p_ps[q][:])

            # ---- AllReduce of p within each batch group ----
            with nc.named_scope(f"l{l}_ar"):
                nc.sync.dma_start(
                    p_loc[l][:, :].rearrange("(n p) d -> p n d", p=P),
                    p_stage[:].rearrange("p (n d) -> p n d", n=nTB))
                nc.gpsimd.collective_compute(
                    "AllReduce", ALU.add, replica_groups=RG,
                    ins=[p_loc[l][:, :]], outs=[p_sum[l][:, :]])

            # ---- phase 4: x = ln(x + p_sum) ----
            with nc.named_scope(f"l{l}_p4"):
                nc.sync.dma_start(
                    p_in[:].rearrange("p (n d) -> p n d", n=nTB),
                    p_sum[l][:, :].rearrange("(n p) d -> p n d", p=P))
                for t in range(nTB):
                    xr = sp2.tile([P, D], F32, tag="lnr", name="xr")
                    nc.vector.tensor_tensor(xr[:], x_f32[t][:],
                                            p_in[:, t * D:(t + 1) * D],
                                            ALU.add)
                    layer_norm(xr[:], [x_f32[t][:], x_bf[t][:]])
                    for d in range(nD):
                        trp = pps.tile([P, P], mm_dt, tag="small", name="trp4")
                        nc.tensor.transpose(trp[:],
                                            x_bf[t][:, d * P:(d + 1) * P],
                                            ident_sb[:])
                        nc.vector.tensor_copy(xT_bf[d][:, t * P:(t + 1) * P],
                                              trp[:])

        # ---- final: out = x @ lm_head (staged in p_stage, single DMA) ----
        with nc.named_scope("final"):
            for t in range(nTB):
                o_ps = pps.tile([P, V], F32, tag="small", name="o_ps")
                for d in range(nD):
                    nc.tensor.matmul(o_ps[:], xT_bf[d][:, t * P:(t + 1) * P],
                                     wlm_sb[d][:],
                                     start=(d == 0), stop=(d == nD - 1))
                nc.vector.tensor_copy(p_stage[:, t * V:(t + 1) * V], o_ps[:])
            nc.sync.dma_start(
                out_d[:, :].rearrange("(n p) v -> p n v", p=P),
                p_stage[:].rearrange("p (n v) -> p n v", n=nTB))

        for cm in reversed(_cms):
            cm.__exit__(None, None, None)
        for f in reversed(_keep):
            f()
        _keep.clear()

    nc.compile()
    return nc


def host_inputs(idx, embed, encoder, encoder_v, decoder, lm_head, cfg,
                mm_dt=BF16):
    """Build the 8 per-core input maps (host-side prep is O(MB) copies)."""
    T, D, N, NL, V = cfg["T"], cfg["D"], cfg["N"], cfg["NL"], cfg["V"]
    NH, B = cfg["NH"], cfg["B"]
    np_mm = np.dtype(mybir.dt.np(mm_dt))

    idx = np.asarray(idx)
    embed = np.asarray(embed, dtype=np.float32)
    encoder = np.asarray(encoder, dtype=np.float32)
    encoder_v = np.asarray(encoder_v, dtype=np.float32)
    decoder = np.asarray(decoder, dtype=np.float32)
    lm_head = np.asarray(lm_head, dtype=np.float32)

    # initial x = ln(embed[idx]) in f32 (cheap: B*T*D)
    e = embed[idx]  # (B, T, D)
    mu = e.mean(-1, keepdims=True)
    var = ((e - mu) ** 2).mean(-1, keepdims=True)
    x0 = ((e - mu) / np.sqrt(var + EPS)).astype(np.float32)

    # rope tables in pair-permuted transposed layout [N/2, T]
    theta = np.float32(2.0 ** 16)
    q = (np.floor(np.arange(N, dtype=np.float32) / 2.0) * 2.0).astype(np.float32)
    freqs = (1.0 / (theta ** (q / np.float32(N))) /
             np.float32(2.0 * math.pi)).astype(np.float32)
    fp = freqs[0::2]  # (N/2,)
    ph = fp[:, None] * np.arange(T, dtype=np.float32)[None, :]
    pm = ((ph % np.float32(1.0)) * np.float32(2.0 * math.pi)).astype(np.float32)
    ctab = np.cos(pm).astype(np_mm)
    stab = np.sin(pm).astype(np_mm)

    perm = np.concatenate([np.arange(0, N, 2), np.arange(1, N, 2)])
    ident = np.eye(P, dtype=np_mm)
    maskt = np.triu(np.ones((P, P), np.float32), k=1).astype(np_mm)  # s < t

    in_maps = []
    for c in range(B * NH):
        b, h = c // NH, c % NH
        in_maps.append({
            "x0": x0[b],
            "wenc": encoder[h][:, perm].astype(np_mm),
            "wencv": encoder_v[h][:, perm].astype(np_mm),
            "wdec": decoder[h * N:(h + 1) * N, :][perm, :].astype(np_mm),
            "wlm": lm_head.astype(np_mm),
            "ctab": ctab,
            "stab": stab,
            "ident": ident,
            "maskt": maskt,
        })
    return in_maps


_NC_CACHE = {}


def _get_nc(cfg_key, cfg, mm_dt, n_cores):
    if cfg_key not in _NC_CACHE:
        _NC_CACHE[cfg_key] = build_nc(cfg, mm_dt=mm_dt, n_cores=n_cores)
    return _NC_CACHE[cfg_key]


def kernel(idx, embed, encoder, encoder_v, decoder, lm_head):
    cfg = FULL_CFG
    NH, B = cfg["NH"], cfg["B"]
    n_cores = B * NH
    in_maps = host_inputs(idx, embed, encoder, encoder_v, decoder, lm_head, cfg)
    nc = _get_nc("full_bf16", cfg, BF16, n_cores)
    res = run_bass_kernel_spmd(nc, in_maps, core_ids=list(range(n_cores)))
    out = np.stack([np.asarray(res.results[b * NH]["out"], dtype=np.float32)
                    for b in range(B)], axis=0)
    return out
